# revision 30
# baseline (speedup 1.0000x reference)
"""DoubleMHAttention on 8 trn2 NeuronCores via Bass/Tile.

Sharding: batch B=4 x head-group G=2 -> 8 cores. Core c handles batch
b=c//2 and heads [4g, 4g+4) with g=c%2. Each core receives its batch's
activations pre-transposed to (C, N) and its head-group's projection
weights pre-transposed to (C, 512), computes 4 heads of the double
attention, and returns attn_p (4,1024,1024) and x_p (1024,512).

Engine split: PE does projections/scores/transposes/x; ACT does only exp
(+one batched run of sqrt) to avoid 1.3us activation-table reloads; DVE
does PSUM evacuations and softmax combines; GpSimd takes the squares and
the exp_cls prescale; SP carries all DMA.
"""

import os
from contextlib import ExitStack

import numpy as np

import concourse.bass as bass
import concourse.mybir as mybir
from concourse.bass_utils import run_bass_kernel_spmd
from concourse.masks import make_identity
from concourse.tile import TileContext, add_dep_helper
from concourse.vector_clock import ScopedClock, VectorClock

P = 128
N = 1024  # sequence length
B = 4  # batch
C = 1024  # d_model
H = 8  # total heads
D = 128  # head dim
NCORES = 8
G = 2  # head groups
NH = H // G  # heads per core
HD = NH * D  # 512 output cols per core
KT = C // P  # 8 contraction tiles
NT = N // P  # 8 sequence tiles
HF = N // 512  # free-dim halves
F32 = mybir.dt.float32
F32R = mybir.dt.float32r
EXP = mybir.ActivationFunctionType.Exp
MUL = mybir.AluOpType.mult
ADD = mybir.AluOpType.add

ACT_NAMES = ("qrT", "krT", "qcT", "kcT", "vT")
W_NAMES = ("wqr", "wkr", "wqc", "wkc", "wv")


class _TC(TileContext):
    """Workaround: this walrus build's CTRL-encoded Drain takes only one sem
    wait, but TileContext's exit drain attaches one per pending processor.
    Carry the waits on individual SP nops first, then drain bare."""

    def _drain_and_barrier(self, tick_clock, wait_clock):
        gc = tick_clock.global_clock
        n = len(gc)
        for p in range(n):
            t = gc[p]
            if t > 0:
                nop_inst = self.nc.sync.nop(nofuse=True, hint=f"drain_wait_p{p}")
                req = ScopedClock(
                    {None: VectorClock([t if i == p else 0 for i in range(n)])}
                )
                wait_clock.add_sem_waits(nop_inst.ins, req)
        self.nc.sync.drain()
        self.nc.all_engine_barrier()
        assert self.sems is not None
        popped = self.nc._tile_sem_poison_stack.pop()
        assert popped is self._sem_poison
        self.nc.clear_and_free_semaphores(list(self.sems.allocated().values()))
        self.nc.all_engine_barrier()


def _split_multi_waits(nc):
    """This walrus build allows at most one sem wait per instruction. Move
    extra waits onto same-engine NOPs inserted just before the instruction
    (engine program order preserves the wait-before-execute semantics)."""
    n_split = 0
    for fn in nc.m.functions:
        for bb in fn.blocks:
            out = []
            for inst in bb.instructions:
                si = inst.sync_info
                if si is not None and len(si.on_wait) > 1:
                    waits = list(si.on_wait)
                    for w_i, w in enumerate(waits[:-1]):
                        nop = mybir.InstNoOp(
                            name=f"{inst.name}_w{w_i}", ins=[], outs=[]
                        )
                        nop.engine = inst.engine
                        nop.sync_info = mybir.SyncInfo(on_wait=[w], on_update=[])
                        out.append(nop)
                        n_split += 1
                    inst.sync_info = mybir.SyncInfo(
                        on_wait=[waits[-1]], on_update=list(si.on_update)
                    )
                out.append(inst)
            bb.instructions = out
    return n_split


def _build_nc():
    nc = bass.Bass()
    acts = {
        nm: nc.dram_tensor(nm, (C, N), F32R, kind="ExternalInput") for nm in ACT_NAMES
    }
    wts = {
        nm: nc.dram_tensor(nm, (C, HD), F32R, kind="ExternalInput") for nm in W_NAMES
    }
    attn_out = nc.dram_tensor("attn_p", (NH, N, N), F32R, kind="ExternalOutput")
    x_out = nc.dram_tensor("x_pT", (HD, N), F32, kind="ExternalOutput")

    with ExitStack() as ctx:
        tc = ctx.enter_context(
            _TC(nc, trace_sim=bool(os.environ.get("TRACE_SIM")))
        )

        const = ctx.enter_context(tc.tile_pool(name="const", bufs=1))
        ident_f = const.tile([P, P], F32, tag="ident_f")
        make_identity(nc, ident_f)
        ident = const.tile([P, P], F32R, tag="ident")
        nc.vector.tensor_copy(ident[:], ident_f[:])
        ones_f32 = const.tile([P, 1], F32, tag="ones_f32")
        nc.vector.memset(ones_f32, 1.0)
        ones_col = const.tile([P, 1], F32R, tag="ones")
        nc.vector.tensor_copy(ones_col[:], ones_f32[:])
        onesr_f32 = const.tile([1, P], F32, tag="onesr_f32")
        nc.vector.memset(onesr_f32, 1.0)
        ones_row = const.tile([1, P], F32R, tag="ones_row")
        nc.vector.tensor_copy(ones_row[:], onesr_f32[:])

        pa = ctx.enter_context(tc.tile_pool(name="pa", bufs=2, space="PSUM"))
        psc = ctx.enter_context(tc.tile_pool(name="psc", bufs=2, space="PSUM"))
        pb = ctx.enter_context(tc.tile_pool(name="pb", bufs=2, space="PSUM"))
        dramp = ctx.enter_context(tc.tile_pool(name="dramp", bufs=4, space="DRAM"))

        projp = ctx.enter_context(tc.tile_pool(name="projp", bufs=1))
        proj = {}  # (tensor, head) -> [128 d, 1024 n] transposed projection

        # ---- Stage 1: the five projections, all heads -------------------
        KG = 4  # act DMA chunks per tensor (k-tiles per chunk = KT // KG)
        with tc.tile_pool(name="actp", bufs=2) as actp, tc.tile_pool(
            name="wp", bufs=2
        ) as wp:
            for t_i, (anm, wnm) in enumerate(zip(ACT_NAMES, W_NAMES)):
                wt = wp.tile([P, KT, HD], F32R, tag="w")
                nc.gpsimd.dma_start(
                    wt[:], wts[wnm].rearrange("(k p) d -> p k d", p=P)
                )
                at = actp.tile([P, KT, N], F32R, tag="act")
                kc = KT // KG
                for kg in range(KG):
                    nc.sync.dma_start(
                        at[:, kg * kc : (kg + 1) * kc, :],
                        acts[anm][kg * kc * P : (kg + 1) * kc * P, :].rearrange(
                            "(k p) n -> p k n", p=P
                        ),
                    )
                for h in range(NH):
                    po = projp.tile([P, N], F32R, tag=f"proj_{t_i}_{h}")
                    proj[(t_i, h)] = po
                    for hf in range(HF):
                        ps = pa.tile([P, 512], F32, tag="mm")
                        for k in range(KT):
                            nc.tensor.matmul(
                                ps[:],
                                lhsT=wt[:, k, h * P : (h + 1) * P],
                                rhs=at[:, k, hf * 512 : (hf + 1) * 512],
                                start=(k == 0),
                                stop=(k == KT - 1),
                            )
                        nc.vector.tensor_copy(po[:, hf * 512 : (hf + 1) * 512], ps[:])

        # ---- Stage 1.5: l2 norms for q/k, all heads ---------------------
        # r = rsqrt(sum_d t^2) as [1, N]. k tensors: scale in place via
        # ones-outer-product broadcast. q tensors: keep r as per-i-tile
        # columns [128, NT] (DRAM bounce) and fold into the exp scale.
        normp = ctx.enter_context(tc.tile_pool(name="normp", bufs=3))
        rowp = ctx.enter_context(tc.tile_pool(name="rowp", bufs=3))
        rqp = ctx.enter_context(tc.tile_pool(name="rqp", bufs=1))
        smallp = ctx.enter_context(tc.tile_pool(name="smallp", bufs=8))

        rq_cols = {}  # (m_i, head) -> [128, NT] per-i-tile rsqrt columns
        sqrt_insts = []
        for h in range(NH):
            for t_i in range(4):
                src = proj[(t_i, h)]
                is_q = t_i in (0, 2)
                rv = rowp.tile([1, N], F32R, tag="rv")
                for hf in range(HF):
                    sq = normp.tile([P, 512], F32R, tag="sq")
                    nc.gpsimd.tensor_mul(
                        sq[:],
                        src[:, hf * 512 : (hf + 1) * 512],
                        src[:, hf * 512 : (hf + 1) * 512],
                    )
                    ss = pa.tile([1, 512], F32, tag="mm")
                    nc.tensor.matmul(
                        ss[:], lhsT=ones_col[:], rhs=sq[:], start=True, stop=True
                    )
                    rec = rowp.tile([1, 512], F32, tag="rec")
                    nc.vector.reciprocal(rec[:], ss[:])
                    sqrt_insts.append(
                        nc.scalar.sqrt(rv[:, hf * 512 : (hf + 1) * 512], rec[:])
                    )
                if is_q:
                    rd = dramp.tile([1, N], F32, tag="rd")
                    nc.sync.dma_start(rd[:], rv[:].bitcast(F32))
                    rqc = rqp.tile(
                        [P, NT], F32, tag=f"rqc_{t_i}_{h}", name=f"rqc_{t_i}_{h}"
                    )
                    nc.sync.dma_start(
                        rqc[:], rd[0, :].rearrange("(t p) -> p t", p=P)
                    )
                    rq_cols[(t_i // 2, h)] = rqc
                else:
                    for hf in range(HF):
                        rb = pa.tile([P, 512], F32, tag="mm")
                        nc.tensor.matmul(
                            rb[:],
                            lhsT=ones_row[:],
                            rhs=rv[:, hf * 512 : (hf + 1) * 512],
                            start=True,
                            stop=True,
                        )
                        nc.vector.tensor_mul(
                            src[:, hf * 512 : (hf + 1) * 512],
                            src[:, hf * 512 : (hf + 1) * 512],
                            rb[:],
                        )

        # ACT runs all sqrts before any exp so the activation table is
        # loaded exactly twice (sqrt once, exp once) instead of thrashing.
        act_marker = nc.scalar.nop(nofuse=True, hint="act_sqrt_exp_split")
        for s_inst in sqrt_insts:
            add_dep_helper(act_marker.ins, s_inst.ins, sync=False,
                           reason="batch sqrts before exps")

        # ---- Stage 2: per-head attention --------------------------------
        vnatp = ctx.enter_context(tc.tile_pool(name="vnatp", bufs=1))
        expp = ctx.enter_context(tc.tile_pool(name="expp", bufs=8))
        tmpp = ctx.enter_context(tc.tile_pool(name="tmpp", bufs=3))
        attnp = ctx.enter_context(tc.tile_pool(name="attnp", bufs=1))
        attnTp = ctx.enter_context(tc.tile_pool(name="attnTp", bufs=1))
        xTp = ctx.enter_context(tc.tile_pool(name="xTp", bufs=1))

        for h in range(NH):
            q_reg, k_reg = proj[(0, h)], proj[(1, h)]
            q_cls, k_cls = proj[(2, h)], proj[(3, h)]
            v_T = proj[(4, h)]

            # v in natural layout [n(j), d] via PE transpose
            vnat = vnatp.tile([P, NT, P], F32R, tag="vnat")
            for jg in range(NT // 4):
                tp = pb.tile([P, 512], F32R, tag="tr")
                for jj in range(4):
                    j = jg * 4 + jj
                    nc.tensor.transpose(
                        tp[:, jj * P : (jj + 1) * P],
                        v_T[:, j * P : (j + 1) * P],
                        ident[:],
                    )
                nc.vector.tensor_copy(vnat[:, jg * 4 : (jg + 1) * 4, :], tp[:])

            # scores -> exp (+row-sum accum) -> softmax combine -> attn out
            attn_grp = {}
            attn_T = [
                attnTp.tile([P, N], F32R, tag=f"attnT_{j}", name=f"attnT_{j}")
                for j in range(NT)
            ]
            for i in range(NT):
                ig, ii = i // 4, i % 4
                if ii == 0:
                    attn_grp[ig] = attnp.tile(
                        [P, 4, N], F32R, tag="attn", name="attn_grp"
                    )
                ex = {}
                acc = {}
                for m_i, (qq, kk) in enumerate(((q_reg, k_reg), (q_cls, k_cls))):
                    ps = psc.tile([P, N], F32, tag="sc")
                    for hf in range(HF):
                        nc.tensor.matmul(
                            ps[:, hf * 512 : (hf + 1) * 512],
                            lhsT=qq[:, i * P : (i + 1) * P],
                            rhs=kk[:, hf * 512 : (hf + 1) * 512],
                            start=True,
                            stop=True,
                        )
                    e = expp.tile([P, N], F32, tag="exp")
                    a = smallp.tile([P, 1], F32, tag="acc")
                    exp_inst = nc.scalar.activation(
                        e[:],
                        ps[:],
                        EXP,
                        scale=rq_cols[(m_i, h)][:, i : i + 1],
                        accum_out=a[:],
                    )
                    add_dep_helper(exp_inst.ins, act_marker.ins, sync=False,
                                   reason="exps after sqrt batch")
                    ex[m_i] = e
                    acc[m_i] = a
                rho = []
                for m_i in range(2):
                    d2 = smallp.tile([P, 1], F32, tag="d2")
                    nc.gpsimd.tensor_scalar_mul(d2[:], acc[m_i][:], 2.0)
                    rh = smallp.tile([P, 1], F32, tag="rho")
                    nc.vector.reciprocal(rh[:], d2[:])
                    rho.append(rh)
                at = attn_grp[ig]
                tmp = tmpp.tile([P, N], F32, tag="tmp")
                nc.gpsimd.tensor_scalar_mul(tmp[:], ex[1][:], rho[1][:])
                nc.vector.scalar_tensor_tensor(
                    at[:, ii, :],
                    ex[0][:],
                    rho[0][:],
                    tmp[:],
                    MUL,
                    ADD,
                )

                if ii == 3:
                    # group of 4 row-blocks complete: DMA out + fold into attn^T
                    nc.sync.dma_start(
                        attn_out[
                            h, ig * 512 : (ig + 1) * 512, :
                        ].rearrange("(q p) n -> p q n", p=P),
                        at[:],
                    )
                    for j in range(NT):
                        tp = pb.tile([P, 512], F32R, tag="tr")
                        for q_i in range(4):
                            nc.tensor.transpose(
                                tp[:, q_i * P : (q_i + 1) * P],
                                at[:, q_i, j * P : (j + 1) * P],
                                ident[:],
                            )
                        dst = attn_T[j][:, ig * 512 : (ig + 1) * 512]
                        if j % 2 == 0:
                            cp = nc.scalar.copy(dst, tp[:])
                            add_dep_helper(cp.ins, act_marker.ins, sync=False,
                                           reason="act copies share exp table")
                        else:
                            nc.vector.tensor_copy(dst, tp[:])

            # x^T[d, i] = sum_j v[j, d]^T-matmul with attn^T[j, i]
            xT = xTp.tile([P, N], F32, tag="xT")
            for hf in range(HF):
                psx = pa.tile([P, 512], F32, tag="mm")
                for j in range(NT):
                    nc.tensor.matmul(
                        psx[:],
                        lhsT=vnat[:, j, :],
                        rhs=attn_T[j][:, hf * 512 : (hf + 1) * 512],
                        start=(j == 0),
                        stop=(j == NT - 1),
                    )
                nc.vector.tensor_copy(xT[:, hf * 512 : (hf + 1) * 512], psx[:])

            nc.sync.dma_start(x_out[h * P : (h + 1) * P, :], xT[:])

    _split_multi_waits(nc)
    return nc


_NC_CACHE = None


def _get_nc():
    global _NC_CACHE
    if _NC_CACHE is None:
        _NC_CACHE = _build_nc()
    return _NC_CACHE


def make_in_maps(query_reg, key_reg, query_cls, key_cls, value,
                 W_q_reg, W_k_reg, W_q_cls, W_k_cls, W_v):
    acts = (query_reg, key_reg, query_cls, key_cls, value)
    ws = (W_q_reg, W_k_reg, W_q_cls, W_k_cls, W_v)
    acts = [np.asarray(a, dtype=np.float32) for a in acts]
    ws = [np.asarray(w, dtype=np.float32) for w in ws]
    actT = {}  # (name, b) -> (C, N)
    for nm, a in zip(ACT_NAMES, acts):
        for b in range(B):
            actT[(nm, b)] = np.ascontiguousarray(a[:, b, :].T)
    wT = {}  # (name, g) -> (C, HD)
    for nm, w in zip(W_NAMES, ws):
        for g in range(G):
            wT[(nm, g)] = np.ascontiguousarray(w[g * HD : (g + 1) * HD, :].T)
    in_maps = []
    for core in range(NCORES):
        b, g = core // G, core % G
        m = {nm: actT[(nm, b)] for nm in ACT_NAMES}
        m.update({nm: wT[(nm, g)] for nm in W_NAMES})
        in_maps.append(m)
    return in_maps


def gather_outputs(results):
    x = np.empty((N, B, C), np.float32)
    attn = np.empty((B, H, N, N), np.float32)
    for core, r in enumerate(results):
        b, g = core // G, core % G
        x[:, b, g * HD : (g + 1) * HD] = r["x_pT"].T
        attn[b, g * NH : (g + 1) * NH] = r["attn_p"]
    return x, attn


def kernel(**inputs):
    in_maps = make_in_maps(**inputs)
    res = run_bass_kernel_spmd(_get_nc(), in_maps, core_ids=list(range(NCORES)))
    return gather_outputs(res.results)


# revision 31
# speedup vs baseline: 334.4596x; 334.4596x over previous
"""DoubleMHAttention on 8 trn2 NeuronCores via Bass/Tile.

Sharding: batch B=4 x head-group G=2 -> 8 cores. Core c handles batch
b=c//2 and heads [4g, 4g+4) with g=c%2. Each core receives its batch's
activations pre-transposed to (C, N) and its head-group's projection
weights pre-transposed to (C, 512), computes 4 heads of the double
attention, and returns attn_p (4,1024,1024) and x_pT (512,1024); the
host assembles the full (N,B,C) x and (B,H,N,N) attn.

Engine split: PE does projections/scores/transposes/x matmuls; ACT does
exp (+one batched run of sqrt — batching avoids 1.3us activation-table
reloads) and half the attn^T evacuations; DVE does the other PSUM
evacuations and the softmax combine; GpSimd takes squares, prescales and
the weight-load DMA queue; SP carries activation-load and output DMA.
"""

import os
from contextlib import ExitStack

import numpy as np

import concourse.bass as bass
import concourse.mybir as mybir
from concourse.bass_utils import run_bass_kernel_spmd
from concourse.masks import make_identity
from concourse.tile import TileContext, add_dep_helper
from concourse.vector_clock import ScopedClock, VectorClock

P = 128
N = 1024  # sequence length
B = 4  # batch
C = 1024  # d_model
H = 8  # total heads
D = 128  # head dim
NCORES = 8
G = 2  # head groups
NH = H // G  # heads per core
HD = NH * D  # 512 output cols per core
KT = C // P  # 8 contraction tiles
NT = N // P  # 8 sequence tiles
HF = N // 512  # free-dim halves
F32 = mybir.dt.float32
F32R = mybir.dt.float32r
EXP = mybir.ActivationFunctionType.Exp
MUL = mybir.AluOpType.mult
ADD = mybir.AluOpType.add

ACT_NAMES = ("qrT", "krT", "qcT", "kcT", "vT")
W_NAMES = ("wqr", "wkr", "wqc", "wkc", "wv")


class _TC(TileContext):
    """Workaround: this walrus build's CTRL-encoded Drain takes only one sem
    wait, but TileContext's exit drain attaches one per pending processor.
    Carry the waits on individual SP nops first, then drain bare."""

    def _drain_and_barrier(self, tick_clock, wait_clock):
        gc = tick_clock.global_clock
        n = len(gc)
        for p in range(n):
            t = gc[p]
            if t > 0:
                nop_inst = self.nc.sync.nop(nofuse=True, hint=f"drain_wait_p{p}")
                req = ScopedClock(
                    {None: VectorClock([t if i == p else 0 for i in range(n)])}
                )
                wait_clock.add_sem_waits(nop_inst.ins, req)
        self.nc.sync.drain()
        self.nc.all_engine_barrier()
        assert self.sems is not None
        popped = self.nc._tile_sem_poison_stack.pop()
        assert popped is self._sem_poison
        self.nc.clear_and_free_semaphores(list(self.sems.allocated().values()))
        self.nc.all_engine_barrier()


def _split_multi_waits(nc):
    """This walrus build allows at most one sem wait per instruction. Move
    extra waits onto same-engine NOPs inserted just before the instruction
    (engine program order preserves the wait-before-execute semantics)."""
    n_split = 0
    for fn in nc.m.functions:
        for bb in fn.blocks:
            out = []
            for inst in bb.instructions:
                si = inst.sync_info
                if si is not None and len(si.on_wait) > 1:
                    waits = list(si.on_wait)
                    for w_i, w in enumerate(waits[:-1]):
                        nop = mybir.InstNoOp(
                            name=f"{inst.name}_w{w_i}", ins=[], outs=[]
                        )
                        nop.engine = inst.engine
                        nop.sync_info = mybir.SyncInfo(on_wait=[w], on_update=[])
                        out.append(nop)
                        n_split += 1
                    inst.sync_info = mybir.SyncInfo(
                        on_wait=[waits[-1]], on_update=list(si.on_update)
                    )
                out.append(inst)
            bb.instructions = out
    return n_split


def _build_nc():
    nc = bass.Bass()
    acts = {
        nm: nc.dram_tensor(nm, (C, N), F32R, kind="ExternalInput") for nm in ACT_NAMES
    }
    wts = {
        nm: nc.dram_tensor(nm, (C, HD), F32R, kind="ExternalInput") for nm in W_NAMES
    }
    attn_out = nc.dram_tensor("attn_p", (NH, N, N), F32R, kind="ExternalOutput")
    x_out = nc.dram_tensor("x_pT", (HD, N), F32, kind="ExternalOutput")

    with ExitStack() as ctx:
        tc = ctx.enter_context(
            _TC(nc, trace_sim=bool(os.environ.get("TRACE_SIM")))
        )

        const = ctx.enter_context(tc.tile_pool(name="const", bufs=1))
        ident_f = const.tile([P, P], F32, tag="ident_f")
        make_identity(nc, ident_f)
        ident = const.tile([P, P], F32R, tag="ident")
        nc.vector.tensor_copy(ident[:], ident_f[:])
        ones_f32 = const.tile([P, 1], F32, tag="ones_f32")
        nc.vector.memset(ones_f32, 1.0)
        ones_col = const.tile([P, 1], F32R, tag="ones")
        nc.vector.tensor_copy(ones_col[:], ones_f32[:])
        onesr_f32 = const.tile([1, P], F32, tag="onesr_f32")
        nc.vector.memset(onesr_f32, 1.0)
        ones_row = const.tile([1, P], F32R, tag="ones_row")
        nc.vector.tensor_copy(ones_row[:], onesr_f32[:])

        pa = ctx.enter_context(tc.tile_pool(name="pa", bufs=2, space="PSUM"))
        psc = ctx.enter_context(tc.tile_pool(name="psc", bufs=2, space="PSUM"))
        pb = ctx.enter_context(tc.tile_pool(name="pb", bufs=2, space="PSUM"))
        dramp = ctx.enter_context(tc.tile_pool(name="dramp", bufs=4, space="DRAM"))

        projp = ctx.enter_context(tc.tile_pool(name="projp", bufs=1))
        proj = {}  # (tensor, head) -> [128 d, 1024 n] transposed projection

        # ---- Stage 1: the five projections, all heads -------------------
        KG = 4  # act DMA chunks per tensor (k-tiles per chunk = KT // KG)
        with tc.tile_pool(name="actp", bufs=2) as actp, tc.tile_pool(
            name="wp", bufs=2
        ) as wp:
            for t_i, (anm, wnm) in enumerate(zip(ACT_NAMES, W_NAMES)):
                wt = wp.tile([P, KT, HD], F32R, tag="w")
                nc.gpsimd.dma_start(
                    wt[:], wts[wnm].rearrange("(k p) d -> p k d", p=P)
                )
                at = actp.tile([P, KT, N], F32R, tag="act")
                kc = KT // KG
                for kg in range(KG):
                    nc.sync.dma_start(
                        at[:, kg * kc : (kg + 1) * kc, :],
                        acts[anm][kg * kc * P : (kg + 1) * kc * P, :].rearrange(
                            "(k p) n -> p k n", p=P
                        ),
                    )
                for h in range(NH):
                    po = projp.tile([P, N], F32R, tag=f"proj_{t_i}_{h}")
                    proj[(t_i, h)] = po
                    for hf in range(HF):
                        ps = pa.tile([P, 512], F32, tag="mm")
                        for k in range(KT):
                            nc.tensor.matmul(
                                ps[:],
                                lhsT=wt[:, k, h * P : (h + 1) * P],
                                rhs=at[:, k, hf * 512 : (hf + 1) * 512],
                                start=(k == 0),
                                stop=(k == KT - 1),
                            )
                        nc.vector.tensor_copy(po[:, hf * 512 : (hf + 1) * 512], ps[:])

        # ---- Stage 1.5: l2 norms for q/k, all heads ---------------------
        # r = rsqrt(sum_d t^2) as [1, N]. k tensors: scale in place via
        # ones-outer-product broadcast. q tensors: keep r as per-i-tile
        # columns [128, NT] (DRAM bounce) and fold into the exp scale.
        normp = ctx.enter_context(tc.tile_pool(name="normp", bufs=3))
        rowp = ctx.enter_context(tc.tile_pool(name="rowp", bufs=3))
        rqp = ctx.enter_context(tc.tile_pool(name="rqp", bufs=1))
        smallp = ctx.enter_context(tc.tile_pool(name="smallp", bufs=8))

        rq_cols = {}  # (m_i, head) -> [128, NT] per-i-tile rsqrt columns
        sqrt_insts = []
        for h in range(NH):
            for t_i in range(4):
                src = proj[(t_i, h)]
                is_q = t_i in (0, 2)
                rv = rowp.tile([1, N], F32R, tag="rv")
                for hf in range(HF):
                    sq = normp.tile([P, 512], F32R, tag="sq")
                    nc.gpsimd.tensor_mul(
                        sq[:],
                        src[:, hf * 512 : (hf + 1) * 512],
                        src[:, hf * 512 : (hf + 1) * 512],
                    )
                    ss = pa.tile([1, 512], F32, tag="mm")
                    nc.tensor.matmul(
                        ss[:], lhsT=ones_col[:], rhs=sq[:], start=True, stop=True
                    )
                    rec = rowp.tile([1, 512], F32, tag="rec")
                    nc.vector.reciprocal(rec[:], ss[:])
                    sqrt_insts.append(
                        nc.scalar.sqrt(rv[:, hf * 512 : (hf + 1) * 512], rec[:])
                    )
                if is_q:
                    rd = dramp.tile([1, N], F32, tag="rd")
                    nc.sync.dma_start(rd[:], rv[:].bitcast(F32))
                    rqc = rqp.tile(
                        [P, NT], F32, tag=f"rqc_{t_i}_{h}", name=f"rqc_{t_i}_{h}"
                    )
                    nc.sync.dma_start(
                        rqc[:], rd[0, :].rearrange("(t p) -> p t", p=P)
                    )
                    rq_cols[(t_i // 2, h)] = rqc
                else:
                    for hf in range(HF):
                        rb = pa.tile([P, 512], F32, tag="mm")
                        nc.tensor.matmul(
                            rb[:],
                            lhsT=ones_row[:],
                            rhs=rv[:, hf * 512 : (hf + 1) * 512],
                            start=True,
                            stop=True,
                        )
                        nc.vector.tensor_mul(
                            src[:, hf * 512 : (hf + 1) * 512],
                            src[:, hf * 512 : (hf + 1) * 512],
                            rb[:],
                        )

        # ACT runs all sqrts before any exp so the activation table is
        # loaded exactly twice (sqrt once, exp once) instead of thrashing.
        act_marker = nc.scalar.nop(nofuse=True, hint="act_sqrt_exp_split")
        for s_inst in sqrt_insts:
            add_dep_helper(act_marker.ins, s_inst.ins, sync=False,
                           reason="batch sqrts before exps")

        # ---- Stage 2: per-head attention --------------------------------
        vnatp = ctx.enter_context(tc.tile_pool(name="vnatp", bufs=1))
        expp = ctx.enter_context(tc.tile_pool(name="expp", bufs=8))
        tmpp = ctx.enter_context(tc.tile_pool(name="tmpp", bufs=3))
        attnp = ctx.enter_context(tc.tile_pool(name="attnp", bufs=1))
        attnTp = ctx.enter_context(tc.tile_pool(name="attnTp", bufs=1))
        xTp = ctx.enter_context(tc.tile_pool(name="xTp", bufs=1))

        for h in range(NH):
            q_reg, k_reg = proj[(0, h)], proj[(1, h)]
            q_cls, k_cls = proj[(2, h)], proj[(3, h)]
            v_T = proj[(4, h)]

            # v in natural layout [n(j), d] via PE transpose
            vnat = vnatp.tile([P, NT, P], F32R, tag="vnat")
            for jg in range(NT // 4):
                tp = pb.tile([P, 512], F32R, tag="tr")
                for jj in range(4):
                    j = jg * 4 + jj
                    nc.tensor.transpose(
                        tp[:, jj * P : (jj + 1) * P],
                        v_T[:, j * P : (j + 1) * P],
                        ident[:],
                    )
                nc.vector.tensor_copy(vnat[:, jg * 4 : (jg + 1) * 4, :], tp[:])

            # scores -> exp (+row-sum accum) -> softmax combine -> attn out
            attn_grp = {}
            attn_T = [
                attnTp.tile([P, N], F32R, tag=f"attnT_{j}", name=f"attnT_{j}")
                for j in range(NT)
            ]
            for i in range(NT):
                ig, ii = i // 4, i % 4
                if ii == 0:
                    attn_grp[ig] = attnp.tile(
                        [P, 4, N], F32R, tag="attn", name="attn_grp"
                    )
                ex = {}
                acc = {}
                for m_i, (qq, kk) in enumerate(((q_reg, k_reg), (q_cls, k_cls))):
                    ps = psc.tile([P, N], F32, tag="sc")
                    for hf in range(HF):
                        nc.tensor.matmul(
                            ps[:, hf * 512 : (hf + 1) * 512],
                            lhsT=qq[:, i * P : (i + 1) * P],
                            rhs=kk[:, hf * 512 : (hf + 1) * 512],
                            start=True,
                            stop=True,
                        )
                    e = expp.tile([P, N], F32, tag="exp")
                    a = smallp.tile([P, 1], F32, tag="acc")
                    exp_inst = nc.scalar.activation(
                        e[:],
                        ps[:],
                        EXP,
                        scale=rq_cols[(m_i, h)][:, i : i + 1],
                        accum_out=a[:],
                    )
                    add_dep_helper(exp_inst.ins, act_marker.ins, sync=False,
                                   reason="exps after sqrt batch")
                    ex[m_i] = e
                    acc[m_i] = a
                rho = []
                for m_i in range(2):
                    d2 = smallp.tile([P, 1], F32, tag="d2")
                    nc.gpsimd.tensor_scalar_mul(d2[:], acc[m_i][:], 2.0)
                    rh = smallp.tile([P, 1], F32, tag="rho")
                    nc.vector.reciprocal(rh[:], d2[:])
                    rho.append(rh)
                at = attn_grp[ig]
                tmp = tmpp.tile([P, N], F32, tag="tmp")
                nc.gpsimd.tensor_scalar_mul(tmp[:], ex[1][:], rho[1][:])
                nc.vector.scalar_tensor_tensor(
                    at[:, ii, :],
                    ex[0][:],
                    rho[0][:],
                    tmp[:],
                    MUL,
                    ADD,
                )

                if ii == 3:
                    # group of 4 row-blocks complete: DMA out + fold into attn^T
                    nc.sync.dma_start(
                        attn_out[
                            h, ig * 512 : (ig + 1) * 512, :
                        ].rearrange("(q p) n -> p q n", p=P),
                        at[:],
                    )
                    for j in range(NT):
                        tp = pb.tile([P, 512], F32R, tag="tr")
                        for q_i in range(4):
                            nc.tensor.transpose(
                                tp[:, q_i * P : (q_i + 1) * P],
                                at[:, q_i, j * P : (j + 1) * P],
                                ident[:],
                            )
                        dst = attn_T[j][:, ig * 512 : (ig + 1) * 512]
                        if j % 2 == 0:
                            cp = nc.scalar.copy(dst, tp[:])
                            add_dep_helper(cp.ins, act_marker.ins, sync=False,
                                           reason="act copies share exp table")
                        else:
                            nc.vector.tensor_copy(dst, tp[:])

            # x^T[d, i] = sum_j v[j, d]^T-matmul with attn^T[j, i]
            xT = xTp.tile([P, N], F32, tag="xT")
            for hf in range(HF):
                psx = pa.tile([P, 512], F32, tag="mm")
                for j in range(NT):
                    nc.tensor.matmul(
                        psx[:],
                        lhsT=vnat[:, j, :],
                        rhs=attn_T[j][:, hf * 512 : (hf + 1) * 512],
                        start=(j == 0),
                        stop=(j == NT - 1),
                    )
                nc.vector.tensor_copy(xT[:, hf * 512 : (hf + 1) * 512], psx[:])

            nc.sync.dma_start(x_out[h * P : (h + 1) * P, :], xT[:])

    _split_multi_waits(nc)
    return nc


_NC_CACHE = None


def _get_nc():
    global _NC_CACHE
    if _NC_CACHE is None:
        _NC_CACHE = _build_nc()
    return _NC_CACHE


def make_in_maps(query_reg, key_reg, query_cls, key_cls, value,
                 W_q_reg, W_k_reg, W_q_cls, W_k_cls, W_v):
    acts = (query_reg, key_reg, query_cls, key_cls, value)
    ws = (W_q_reg, W_k_reg, W_q_cls, W_k_cls, W_v)
    acts = [np.asarray(a, dtype=np.float32) for a in acts]
    ws = [np.asarray(w, dtype=np.float32) for w in ws]
    actT = {}  # (name, b) -> (C, N)
    for nm, a in zip(ACT_NAMES, acts):
        for b in range(B):
            actT[(nm, b)] = np.ascontiguousarray(a[:, b, :].T)
    wT = {}  # (name, g) -> (C, HD)
    for nm, w in zip(W_NAMES, ws):
        for g in range(G):
            wT[(nm, g)] = np.ascontiguousarray(w[g * HD : (g + 1) * HD, :].T)
    in_maps = []
    for core in range(NCORES):
        b, g = core // G, core % G
        m = {nm: actT[(nm, b)] for nm in ACT_NAMES}
        m.update({nm: wT[(nm, g)] for nm in W_NAMES})
        in_maps.append(m)
    return in_maps


def gather_outputs(results):
    x = np.empty((N, B, C), np.float32)
    attn = np.empty((B, H, N, N), np.float32)
    for core, r in enumerate(results):
        b, g = core // G, core % G
        x[:, b, g * HD : (g + 1) * HD] = r["x_pT"].T
        attn[b, g * NH : (g + 1) * NH] = r["attn_p"]
    return x, attn


def kernel(**inputs):
    in_maps = make_in_maps(**inputs)
    res = run_bass_kernel_spmd(_get_nc(), in_maps, core_ids=list(range(NCORES)))
    return gather_outputs(res.results)
